# revision 39
# baseline (speedup 1.0000x reference)
"""DDiT block kernel for 8 Trainium2 NeuronCores.

Sharding: core = (batch b = core//2, seq half = core%2). Each core computes
adaLN, LN1 for all 2048 tokens of its batch, q for its own 1024 tokens,
k/v for all 2048 (redundant compute instead of a collective), rotary,
non-causal attention for its 1024 queries, out-proj, LN2, MLP.
All activations live in feature-on-partition layout; the host pre-transposes
x / weights and re-assembles the output.

v12 design (~484us; v7 baseline was ~511us):
  - fp8(e4m3) attention path: W_qkv(x64)/W_v(x32)/W_out(x64) quantized on
    the host, LN1 output h in fp8, DoubleRow matmuls pair two 128-row
    contraction tiles per instruction (2x contraction throughput) for
    q/k/v projections, AV, and out-proj. Scores stay bf16 (64-contraction,
    row-pair tile_position). Scale bookkeeping: scores come out x4096
    (folded into the exp constants), AV output is 32*o (the fp8 sweet
    spot for out-proj), out-proj result is x2048 (1/2048 folded into the
    adaLN msa gate). MLP stays bf16: fp8 there costs ~1.7e-2 rel err.
  - B/C: all four LN1-stat chains first (their ACT/DVE tails overlap the
    adaLN phase nested underneath and the projection matmuls), then
    applies, q/k/v + rope for BOTH halves. Rope-swap DMAs ride the idle
    gpsimd queue.
  - window: pure scores->exp->AV pipeline. psS bufs=3 (12KB) + psO (4KB)
    fill PSUM exactly; no passes/DRAM spills. exp alternates per kc
    between ACT (true exp -> fp8) and DVE (Schraudolph: uint8(A*x+B)
    bitcast to fp8e4; negative scores saturate to 0 = exp underflow).
    The pipeline pace is the cycle exp -> scores(+3) -> exp (~1.6us per
    kc-pair, semaphore hops included); E pairs [P,2(hh),2(kc),512] u8
    feed DoubleRow AV; vA pairs are [P,2,H,80] fp8 (pair stride and head
    offsets 16B-aligned for dual-fp8 LDWEIGHTS, col 64 = ones row giving
    the softmax denominator, 65..79 zero pad).
  - finalize(p) is emitted before p+1's first scores so its PSUM-releasing
    copies (ov: ACT hh0 / DVE hh1) land at the front of both engine
    queues; denominator row must be DMA'd to a partition-0 tile before
    reciprocal_approx_fast (ucode reads garbage at partition offset 64).
  - oTs is split per query half so the tail's out-proj(0) does not wait
    on the last finalize; the residual x loads as one [P,DK,2,512] tile
    dispatched at window start (scalar queue).
  - tail: op0, ln2(0)-stats, op1 (hides the ln2(0) ACT-sqrt chain),
    ln2(0)-apply, mlp1(0), ln2(1) (hidden under mlp1(0)), mlp1(1),
    mlp2(0), mlp2(1). Two ACT table switches total (exp->sqrt->gelu).
"""

import numpy as np
import sys

sys.path.insert(0, "/opt/trn_rl_repo")

B, S, D, H, DH = 4, 2048, 768, 12, 64
COND, MLP = 128, 3072
EPS = 1e-5
P = 128
SH = S // 2          # tokens per core (1024)
DK = D // P          # 6 feature chunks
MK = MLP // P        # 24 mlp chunks
KC = S // P          # 16 key blocks
N_CORES = 8

_prog_cache = {}


def _build_program():
    import concourse.tile as tile
    from concourse import bacc
    import concourse.mybir as mybir
    from contextlib import ExitStack

    f32 = mybir.dt.float32
    bf16 = mybir.dt.bfloat16
    fp8 = mybir.dt.float8e4
    u8 = mybir.dt.uint8
    AF = mybir.ActivationFunctionType
    OP = mybir.AluOpType
    MM8 = mybir.MatmulPerfMode.DoubleRow
    # q/k/v weights are scaled x64/x32 on the host for fp8; scores come out
    # x4096. Schraudolph exp2-bit-trick: uint8(A*x + B) bitcast fp8e4
    # approximates exp(x*0.125/4096) (head-scale and fp8 descale folded in).
    SCH_A = 8.0 * 1.4426950408889634 * 0.125 / 4096.0
    SCH_B = 7.0 * 8.0 - 0.344
    EXP_SC = 0.125 / 4096.0

    nc = bacc.Bacc("TRN2", target_bir_lowering=False, debug=False,
                   enable_asserts=False, num_devices=N_CORES)

    # ---- DRAM I/O (per-core shapes) ----
    xT_d = nc.dram_tensor("xT", [D, S], f32, kind="ExternalInput").ap()
    xT16_d = nc.dram_tensor("xT16", [P, DK, S], bf16, kind="ExternalInput").ap()
    c_d = nc.dram_tensor("cT", [COND, 1], f32, kind="ExternalInput").ap()
    cos_d = nc.dram_tensor("cos4", [P, S], bf16, kind="ExternalInput").ap()
    sin_d = nc.dram_tensor("sin4", [P, S], bf16, kind="ExternalInput").ap()
    wada_d = nc.dram_tensor("WadaT", [COND, 6 * D], bf16, kind="ExternalInput").ap()
    bada_d = nc.dram_tensor("badaT", [P, 36], f32, kind="ExternalInput").ap()
    ln1w_d = nc.dram_tensor("ln1wT", [P, DK], f32, kind="ExternalInput").ap()
    ln2w_d = nc.dram_tensor("ln2wT", [P, DK], f32, kind="ExternalInput").ap()
    wqk_d = nc.dram_tensor("WqkB", [2 * DK, P, DK, P], fp8, kind="ExternalInput").ap()
    wv_d = nc.dram_tensor("WvR", [D, D], fp8, kind="ExternalInput").ap()
    wout_d = nc.dram_tensor("WoB", [DK, P, DK, P], fp8, kind="ExternalInput").ap()
    w1_d = nc.dram_tensor("W1B", [MK, P, DK, P], bf16, kind="ExternalInput").ap()
    b1_d = nc.dram_tensor("b1T", [P, MK], f32, kind="ExternalInput").ap()
    w2_d = nc.dram_tensor("W2B", [DK, P, MK, P], bf16, kind="ExternalInput").ap()
    b2_d = nc.dram_tensor("b2T", [P, DK], f32, kind="ExternalInput").ap()
    out_d = nc.dram_tensor("outT", [D, SH], f32, kind="ExternalOutput").ap()

    xT3 = xT_d.rearrange("(a p) n -> p a n", p=P)          # [128, 6, 2048]

    with tile.TileContext(nc) as tc, ExitStack() as ctx:
        base = ctx.enter_context(tc.tile_pool(name="base", bufs=1))
        wpool = ctx.enter_context(tc.tile_pool(name="wpool", bufs=4))
        stat = ctx.enter_context(tc.tile_pool(name="stat", bufs=1))
        bcast = ctx.enter_context(tc.tile_pool(name="bcast", bufs=4))
        sqp = ctx.enter_context(tc.tile_pool(name="sqp", bufs=2))
        rp = ctx.enter_context(tc.tile_pool(name="rope", bufs=1))

        ada = base.tile([P, 36], f32, name="ada")
        ln1s = base.tile([P, DK], f32, name="ln1s")
        ln2s = base.tile([P, DK], f32, name="ln2s")
        ones = base.tile([P, 1], bf16, name="ones")
        nc.vector.memset(ones[:], 1.0)
        epsT = base.tile([1, 1], f32, name="epsT")
        nc.vector.memset(epsT[:], EPS)
        b1s = base.tile([P, MK], f32, name="b1s")
        b2s = base.tile([P, DK], f32, name="b2s")
        cosT = base.tile([P, S], bf16, name="cosT")
        sinT = base.tile([P, S], bf16, name="sinT")

        # cT + weight DMAs kick off first (scalar queue)
        cT = base.tile([COND, 1], f32, name="cT")
        nc.scalar.dma_start(cT[:], c_d[:, :])
        cT16 = base.tile([COND, 1], bf16, name="cT16")
        nc.vector.tensor_copy(cT16[:], cT[:])

        def ln_stats(psp, ps_tag, src_chunk, use_act=True, sqrt_dve=False):
            """Sums/var/rstd for 512 columns; returns (A128, B128) bf16
            broadcast tiles (rstd and mean)."""
            ps = psp.tile([P, 512], f32, tag=ps_tag, name="lnps")
            for k in range(DK):
                x16 = src_chunk(k)
                sq = sqp.tile([P, 512], bf16, tag="sq", name="sq")
                if use_act:
                    nc.scalar.activation(sq[:], x16[:], AF.Square)
                else:
                    nc.vector.tensor_mul(sq[:], x16[:], x16[:])
                nc.tensor.matmul(ps[0:1, :], ones[:], x16[:],
                                 start=(k == 0), stop=(k == DK - 1))
                nc.tensor.matmul(ps[32:33, :], ones[:], sq[:],
                                 start=(k == 0), stop=(k == DK - 1))
            mean = stat.tile([1, 512], f32, tag="mean", name="mean")
            nc.vector.tensor_scalar_mul(mean[:], ps[0:1, :], 1.0 / D)
            var = stat.tile([1, 512], f32, tag="var", name="var")
            nc.vector.tensor_scalar_mul(var[:], ps[32:33, :], 1.0 / D)
            aux = stat.tile([1, 512], f32, tag="aux", name="aux")
            nc.vector.tensor_mul(aux[:], mean[:], mean[:])
            nc.vector.tensor_sub(var[:], var[:], aux[:])
            r0 = stat.tile([1, 512], f32, tag="r0", name="r0")
            if not sqrt_dve:
                sd = stat.tile([1, 512], f32, tag="aux", name="sd")
                nc.scalar.activation(sd[:], var[:], AF.Sqrt, bias=epsT[:])
                nc.vector.reciprocal_approx_fast(out=r0[:], in_=sd[:])
            else:
                # rsqrt via clamped Newton from a constant seed (no ACT
                # sqrt -> no activation-table switches near gelu phases)
                vc = stat.tile([1, 512], f32, tag="vc", name="vc")
                nc.vector.tensor_scalar(vc[:], var[:], 10.0, EPS,
                                        OP.min, OP.add)
                nc.vector.memset(r0[:], 0.4)
                nt = stat.tile([1, 512], f32, tag="nt", name="nt")
                for _ in range(5):
                    nc.vector.tensor_mul(nt[:], r0[:], r0[:])
                    nc.vector.tensor_mul(nt[:], nt[:], vc[:])
                    nc.vector.tensor_scalar(nt[:], nt[:], -0.5, 1.5,
                                            OP.mult, OP.add)
                    nc.vector.tensor_mul(r0[:], r0[:], nt[:])
            rb16 = stat.tile([1, 512], bf16, tag="rb16", name="rb16")
            nc.vector.tensor_copy(rb16[:], r0[:])
            mb16 = stat.tile([1, 512], bf16, tag="mb16", name="mb16")
            nc.vector.tensor_copy(mb16[:], mean[:])
            A128 = bcast.tile([P, 512], bf16, tag="A128", name="A128")
            B128 = bcast.tile([P, 512], bf16, tag="B128", name="B128")
            nc.gpsimd.partition_broadcast(A128[:], rb16[:])
            nc.gpsimd.partition_broadcast(B128[:], mb16[:])
            return A128, B128

        def ln_apply(src_chunk, A128, B128, scale_cols, shift_col0, dst_chunk,
                     use_act=True):
            for k in range(DK):
                t2 = sqp.tile([P, 512], bf16, tag="t2", name="t2")
                nc.vector.tensor_sub(t2[:], src_chunk(k), B128[:])
                nc.vector.tensor_mul(t2[:], t2[:], A128[:])
                if use_act:
                    nc.scalar.activation(
                        dst_chunk(k), t2[:], AF.Identity,
                        bias=ada[:, shift_col0 + k:shift_col0 + k + 1],
                        scale=scale_cols[:, k:k + 1])
                else:
                    nc.vector.tensor_scalar(
                        dst_chunk(k), t2[:], scale_cols[:, k:k + 1],
                        ada[:, shift_col0 + k:shift_col0 + k + 1],
                        OP.mult, OP.add)

        def ln_block(psp, ps_tag, src_chunk, scale_cols, shift_col0, dst_chunk,
                     use_act=True, sqrt_dve=False):
            A128, B128 = ln_stats(psp, ps_tag, src_chunk, use_act=use_act,
                                  sqrt_dve=sqrt_dve)
            ln_apply(src_chunk, A128, B128, scale_cols, shift_col0, dst_chunk,
                     use_act=use_act)

        with tc.tile_pool(name="efgA", bufs=1) as efgA:
            # ======== q/k/v outputs (live through attention) ========
            with tc.tile_pool(name="qkv_out", bufs=1) as qko:
                qT = [qko.tile([P, SH], bf16, name=f"qT{m}") for m in range(DK)]
                kpair = [qko.tile([P, S], bf16, name=f"kp{m}")
                         for m in range(DK)]
                # vA pairs for DoubleRow AV: [keys, kc-pair, head, DH+pad16]
                vA8 = [qko.tile([P, 2, H, 80], fp8, name=f"vA{tp}")
                       for tp in range(KC // 2)]
                wvT = qko.tile([P, DK, D], fp8, name="wvT")

                def rope_swap(sw, src, n):
                    # gpsimd queue: keeps the 48 dispatches off the busy
                    # sync queue in B/C
                    nc.gpsimd.dma_start(sw[0:32, 0:n], src[32:64, 0:n])
                    nc.gpsimd.dma_start(sw[32:64, 0:n], src[0:32, 0:n])
                    nc.gpsimd.dma_start(sw[64:96, 0:n], src[96:128, 0:n])
                    nc.gpsimd.dma_start(sw[96:128, 0:n], src[64:96, 0:n])

                def rope_q(m):
                    sw = rp.tile([P, SH], bf16, tag="qsw", name="qsw")
                    t = qT[m]
                    rope_swap(sw, t[:, 0:SH], SH)
                    nc.vector.tensor_mul(t[:], t[:], cosT[:, 0:SH])
                    nc.vector.tensor_mul(sw[:], sw[:], sinT[:, 0:SH])
                    nc.vector.tensor_add(t[:], t[:], sw[:])

                def rope_k(m, b2):
                    sl = slice(b2 * SH, b2 * SH + SH)
                    sw = rp.tile([P, SH], bf16, tag="ksw", name="ksw")
                    t = kpair[m]
                    rope_swap(sw, t[:, sl], SH)
                    nc.vector.tensor_mul(t[:, sl], t[:, sl], cosT[:, sl])
                    nc.vector.tensor_mul(sw[:], sw[:], sinT[:, sl])
                    nc.vector.tensor_add(t[:, sl], t[:, sl], sw[:])

                def v_proj(t, hbsrc):
                    """v for token block t (128 tokens) from hb half tiles."""
                    ps = psQ.tile([P, 2, 512], f32, tag="mm", name="ps_v")
                    tl = t % 8
                    tp, sub = t // 2, t % 2
                    for j in range(3):
                        lhs = hbsrc[tl // 4][:, 2 * j:2 * j + 2,
                                             (tl % 4) * P:(tl % 4 + 1) * P]
                        nc.tensor.matmul(ps[:, 0, :], lhs,
                                         wvT[:, 2 * j:2 * j + 2, 0:512],
                                         start=(j == 0), stop=(j == 2),
                                         perf_mode=MM8)
                        nc.tensor.matmul(ps[:, 1, 0:256], lhs,
                                         wvT[:, 2 * j:2 * j + 2, 512:768],
                                         start=(j == 0), stop=(j == 2),
                                         perf_mode=MM8)
                    nc.scalar.copy(
                        vA8[tp][:, sub, 0:8, 0:DH],
                        ps[:, 0, :].rearrange("p (h d) -> p h d", d=DH))
                    nc.vector.tensor_copy(
                        vA8[tp][:, sub, 8:H, 0:DH],
                        ps[:, 1, 0:256].rearrange("p (h d) -> p h d", d=DH))
                    nc.vector.memset(vA8[tp][:, sub, :, DH:DH + 1], 1.0)
                    nc.vector.memset(vA8[tp][:, sub, :, DH + 1:80], 0.0)

                # ==== Phase B/C: LN1 + q/k/v + rope for both halves ====
                with tc.tile_pool(name="phbc", bufs=4) as phbc, \
                     tc.tile_pool(name="hbp", bufs=4) as hbp, \
                     tc.tile_pool(name="psLN", bufs=2, space="PSUM") as psLN, \
                     tc.tile_pool(name="psQ", bufs=2, space="PSUM") as psQ:
                    adaw_ctx = tc.tile_pool(name="adaw", bufs=1)
                    adaw = adaw_ctx.__enter__()
                    wt = adaw.tile([COND, 6 * D], bf16, name="wadaT")
                    nc.scalar.dma_start(wt[:], wada_d[:, :])
                    # all 4 LN1 stats first; their ACT/DVE chains overlap
                    # phase A and the projection matmuls below
                    xbs, ABs = [], []
                    for i in range(4):
                        xb = phbc.tile([P, DK, 512], bf16, tag="xb",
                                       name=f"xb{i}")
                        nc.sync.dma_start(xb[:], xT16_d[:, :, i * 512:
                                                        i * 512 + 512])
                        xbs.append(xb)
                        ABs.append(ln_stats(psLN, "lnps",
                                            lambda k, xb=xb: xb[:, k, :]))
                        if i == 0:
                            nc.scalar.dma_start(cosT[:], cos_d[:, :])
                            nc.scalar.dma_start(sinT[:], sin_d[:, :])
                            nc.scalar.dma_start(b1s[:], b1_d[:, :])
                            nc.scalar.dma_start(b2s[:], b2_d[:, :])

                    # ---- Phase A: adaLN modulation (hidden under LN chains)
                    with tc.tile_pool(name="psE", bufs=2, space="PSUM") as psE:
                        for j4 in range(9):
                            ps = psE.tile([P, 4], f32, tag="mm", name="ps_ada")
                            for j in range(4):
                                nc.tensor.matmul(ps[:, j:j + 1],
                                                 wt[:, (4 * j4 + j) * P:
                                                     (4 * j4 + j + 1) * P],
                                                 cT16[:],
                                                 start=True, stop=True)
                            nc.vector.tensor_copy(ada[:, 4 * j4:4 * j4 + 4],
                                                  ps[:])
                        badaT = base.tile([P, 36], f32, name="badaT")
                        nc.scalar.dma_start(badaT[:], bada_d[:, :])
                        nc.vector.tensor_add(ada[:], ada[:], badaT[:])
                        nc.vector.tensor_scalar_add(ada[:, 6:12],
                                                    ada[:, 6:12], 1.0)
                        nc.vector.tensor_scalar_add(ada[:, 24:30],
                                                    ada[:, 24:30], 1.0)
                        lw = base.tile([P, DK], f32, name="lnw1")
                        nc.scalar.dma_start(lw[:], ln1w_d[:, :])
                        nc.vector.tensor_mul(ln1s[:], lw[:], ada[:, 6:12])
                        lw2 = base.tile([P, DK], f32, name="lnw2")
                        nc.scalar.dma_start(lw2[:], ln2w_d[:, :])
                        nc.vector.tensor_mul(ln2s[:], lw2[:], ada[:, 24:30])
                        nc.vector.tensor_scalar_mul(ada[:, 12:18],
                                                    ada[:, 12:18],
                                                    1.0 / 2048.0)
                    adaw_ctx.__exit__(None, None, None)

                    # applies for the own half, then projections (the b2=1
                    # applies run under the q/k matmuls)
                    hb, hb1 = [], []
                    for i in range(2):
                        hbt = hbp.tile([P, DK, 512], fp8, tag="hb",
                                       name=f"hb{i}")
                        hb.append(hbt)
                        ln_apply(lambda k, xb=xbs[i]: xb[:, k, :], ABs[i][0],
                                 ABs[i][1], ln1s, 0,
                                 lambda k, hbt=hbt: hbt[:, k, :])
                    for i in range(2):
                        hbt = hbp.tile([P, DK, 512], fp8, tag="hb",
                                       name=f"hb1_{i}")
                        hb1.append(hbt)
                        ln_apply(lambda k, xb=xbs[2 + i]: xb[:, k, :],
                                 ABs[2 + i][0], ABs[2 + i][1], ln1s, 0,
                                 lambda k, hbt=hbt: hbt[:, k, :])
                    for is_k, wblk0 in ((0, 0), (1, DK)):
                        for m in range(DK):
                            w6 = wpool.tile([P, DK, P], fp8, tag="w6",
                                            name="w6")
                            nc.sync.dma_start(w6[:], wqk_d[wblk0 + m])
                            ps = psQ.tile([P, 2, 512], f32, tag="mm",
                                          name="ps_qk")
                            for j in range(3):
                                for i in range(2):
                                    nc.tensor.matmul(
                                        ps[:, i, :],
                                        w6[:, 2 * j:2 * j + 2, :],
                                        hb[i][:, 2 * j:2 * j + 2, :],
                                        start=(j == 0), stop=(j == 2),
                                        perf_mode=MM8)
                            if is_k:
                                nc.scalar.copy(
                                    kpair[m][:, 0:SH],
                                    ps[:].rearrange("p a n -> p (a n)"))
                            else:
                                nc.vector.tensor_copy(
                                    qT[m][:, 0:SH],
                                    ps[:].rearrange("p a n -> p (a n)"))
                            if is_k:
                                rope_k(m, 0)
                            else:
                                rope_q(m)
                        if is_k:
                            nc.sync.dma_start(
                                wvT[:],
                                wv_d.rearrange("(a p) n -> p a n", p=P))
                    for t in range(SH // P):
                        v_proj(t, hb)

                    # ---- other half (b2=1): k/v proj + rope ----
                    for m in range(DK):
                        w6 = wpool.tile([P, DK, P], fp8, tag="w6", name="w6")
                        nc.sync.dma_start(w6[:], wqk_d[DK + m])
                        ps = psQ.tile([P, 2, 512], f32, tag="mm", name="ps_k1")
                        for j in range(3):
                            for i in range(2):
                                nc.tensor.matmul(ps[:, i, :],
                                                 w6[:, 2 * j:2 * j + 2, :],
                                                 hb1[i][:, 2 * j:2 * j + 2, :],
                                                 start=(j == 0), stop=(j == 2),
                                                 perf_mode=MM8)
                        nc.scalar.copy(
                            kpair[m][:, SH:S],
                            ps[:].rearrange("p a n -> p (a n)"))
                        rope_k(m, 1)
                    for t in range(8, KC):
                        v_proj(t, hb1)

                # ==== Phase D: attention (pure pipeline) ====
                xoall = efgA.tile([P, DK, 2, 512], f32, name="xoall")
                nc.scalar.dma_start(
                    xoall[:],
                    xT3[:, :, 0:SH].rearrange("p a (b n) -> p a b n", n=512))
                with tc.tile_pool(name="attn_sb", bufs=4) as asb, \
                     tc.tile_pool(name="fin", bufs=2) as fin, \
                     tc.tile_pool(name="fin1", bufs=1) as fin1, \
                     tc.tile_pool(name="psS", bufs=3, space="PSUM") as psS, \
                     tc.tile_pool(name="psO", bufs=1, space="PSUM") as psO:
                    oTs0 = efgA.tile([P, DK, 512], fp8, name="oTs0")
                    oTs1 = efgA.tile([P, DK, 512], fp8, name="oTs1")
                    oTsq = [oTs0, oTs1]

                    def emit_scores(p, qb, kc, Ep):
                        sg = psS.tile([P, 2, 512], f32, tag="sg", name="sg")
                        qsl = slice(qb * 512, qb * 512 + 512)
                        for hh in range(2):
                            r0_, r1_ = 64 * hh, 64 * hh + 64
                            nc.tensor.matmul(
                                sg[:, hh, :],
                                kpair[p][r0_:r1_, kc * P:(kc + 1) * P],
                                qT[p][r0_:r1_, qsl], start=True, stop=True)
                        # alternate exp engine per kc: consecutive exps run
                        # concurrently on ACT and DVE.
                        out_ap = Ep[:, :, kc % 2, :]
                        if kc % 2 == 0:
                            nc.scalar.activation(out_ap.bitcast(fp8), sg[:],
                                                 AF.Exp, scale=EXP_SC)
                        else:
                            nc.vector.tensor_scalar(out_ap, sg[:], SCH_A,
                                                    SCH_B, OP.mult, OP.add)

                    def emit_av(p, qb, kcp, Ep, oags):
                        for hh in range(2):
                            nc.tensor.matmul(oags[hh][:],
                                             vA8[kcp][:, :, 2 * p + hh, :],
                                             Ep[:, hh, :, :].bitcast(fp8),
                                             start=(kcp == 0),
                                             stop=(kcp == KC // 2 - 1),
                                             perf_mode=MM8)

                    def emit_finalize(p, qb, oags):
                        oTs = oTsq[qb]
                        ov = fin.tile([DH + 1, 2, 512], f32, tag="ov",
                                      name="ov")
                        nc.scalar.copy(ov[:, 0, :], oags[0][0:DH + 1, :])
                        nc.vector.tensor_copy(ov[:, 1, :],
                                              oags[1][0:DH + 1, :])
                        dn = fin1.tile([1, 2, 512], f32, tag="dn", name="dn")
                        nc.sync.dma_start(dn[:], ov[DH:DH + 1, :, :])
                        rc = fin1.tile([1, 2, 512], f32, tag="rc", name="rc")
                        nc.vector.reciprocal_approx_fast(out=rc[:], in_=dn[:])
                        rcb = fin1.tile([1, 2, 512], bf16, tag="rcb",
                                        name="rcb")
                        nc.vector.tensor_copy(rcb[:], rc[:])
                        rb = fin1.tile([DH, 2, 512], bf16, tag="rb", name="rb")
                        nc.gpsimd.partition_broadcast(rb[:], rcb[:])
                        nc.vector.tensor_mul(oTs[0:DH, p, :],
                                             ov[0:DH, 0, :], rb[:, 0, :])
                        ot = fin1.tile([DH, 512], fp8, tag="ot", name="ot")
                        nc.vector.tensor_mul(ot[:], ov[0:DH, 1, :],
                                             rb[:, 1, :])
                        nc.sync.dma_start(oTs[DH:P, p, :], ot[:])

                    prevfin = None
                    pend = None
                    for qb in range(2):
                        for p in range(H // 2):
                            oags = [psO.tile([80, 512], f32,
                                             tag=f"oag{hh}", name=f"oag{hh}")
                                    for hh in range(2)]
                            if prevfin is not None:
                                emit_finalize(*prevfin)
                                prevfin = None
                            for kcp in range(KC // 2):
                                Ep = asb.tile([P, 2, 2, 512], u8, tag="E",
                                              name="E")
                                emit_scores(p, qb, 2 * kcp, Ep)
                                emit_scores(p, qb, 2 * kcp + 1, Ep)
                                if pend is not None:
                                    emit_av(*pend)
                                pend = (p, qb, kcp, Ep, oags)
                            prevfin = (p, qb, oags)
                    emit_av(*pend)
                    emit_finalize(*prevfin)

            # ======== EFG tail ========
            with tc.tile_pool(name="efgB", bufs=1) as efgB, \
                 tc.tile_pool(name="mlp_tmp", bufs=2) as mt, \
                 tc.tile_pool(name="w24p", bufs=3) as w24p, \
                 tc.tile_pool(name="psM", bufs=2, space="PSUM") as psM:
                x1_0 = efgB.tile([P, DK, 512], bf16, name="x1_0")
                h2_0 = efgB.tile([P, DK, 512], bf16, name="h2_0")
                x1_1 = efgB.tile([P, DK, 512], bf16, name="x1_1")
                h2_1 = efgB.tile([P, DK, 512], bf16, name="h2_1")
                m16_0 = efgB.tile([P, MK, 512], bf16, name="m16_0")
                m16_1 = efgB.tile([P, MK, 512], bf16, name="m16_1")

                def op_unit(ihalf, m, x1t):
                    w6 = wpool.tile([P, DK, P], fp8, tag="w6o", name="w6o")
                    nc.sync.dma_start(w6[:], wout_d[m])
                    ps = psM.tile([P, 512], f32, tag="mm2", name="ps_o")
                    for j in range(3):
                        nc.tensor.matmul(ps[:],
                                         w6[:, 2 * j:2 * j + 2, :],
                                         oTsq[ihalf][:, 2 * j:2 * j + 2, :],
                                         start=(j == 0), stop=(j == 2),
                                         perf_mode=MM8)
                    nc.vector.scalar_tensor_tensor(
                        x1t[:, m, :], ps[:], ada[:, 12 + m:13 + m],
                        xoall[:, m, ihalf, :], OP.mult, OP.add)

                def mlp1_unit(m, h2t, m16t):
                    w6 = wpool.tile([P, DK, P], bf16, tag="w6m", name="w6m")
                    nc.sync.dma_start(w6[:], w1_d[m])
                    ps = psM.tile([P, 512], f32, tag="mm2", name="ps_m")
                    for k in range(DK):
                        nc.tensor.matmul(ps[:], w6[:, k, :], h2t[:, k, :],
                                         start=(k == 0), stop=(k == DK - 1))
                    nc.scalar.activation(m16t[:, m, :], ps[:],
                                         AF.Gelu_apprx_tanh,
                                         bias=b1s[:, m:m + 1])

                def mlp2_half(m, i, m16t, x1t):
                    isl = slice(i * 512, i * 512 + 512)
                    w24 = w24p.tile([P, MK, P], bf16, tag="w24", name="w24")
                    nc.sync.dma_start(w24[:], w2_d[m])
                    ps = psM.tile([P, 512], f32, tag="mm2", name="ps_y")
                    for k in range(MK):
                        nc.tensor.matmul(ps[:], w24[:, k, :], m16t[:, k, :],
                                         start=(k == 0), stop=(k == MK - 1))
                    yt = mt.tile([P, 512], f32, tag="yt", name="yt")
                    nc.vector.tensor_scalar(yt[:], ps[:], b2s[:, m:m + 1],
                                            ada[:, 30 + m:31 + m],
                                            OP.add, OP.mult)
                    nc.vector.tensor_add(yt[:], yt[:], x1t[:, m, :])
                    nc.sync.dma_start(out_d[m * P:(m + 1) * P, isl], yt[:])

                for m in range(DK):
                    op_unit(0, m, x1_0)
                AB0 = ln_stats(psM, "lnps2", lambda k: x1_0[:, k, :],
                               use_act=False)
                for m in range(DK):
                    op_unit(1, m, x1_1)
                ln_apply(lambda k: x1_0[:, k, :], AB0[0], AB0[1], ln2s, 18,
                         lambda k: h2_0[:, k, :], use_act=True)
                AB1 = ln_stats(psM, "lnps2", lambda k: x1_1[:, k, :],
                               use_act=False)
                for m in range(MK):
                    mlp1_unit(m, h2_0, m16_0)
                ln_apply(lambda k: x1_1[:, k, :], AB1[0], AB1[1], ln2s, 18,
                         lambda k: h2_1[:, k, :], use_act=True)
                for m in range(MK):
                    mlp1_unit(m, h2_1, m16_1)
                for m in range(DK):
                    mlp2_half(m, 0, m16_0, x1_0)
                for m in range(DK):
                    mlp2_half(m, 1, m16_1, x1_1)

    nc.compile()
    return nc


def _host_prep(inputs):
    """Build per-core in_maps (host-side sharding + layout transforms)."""
    import ml_dtypes
    bf16 = ml_dtypes.bfloat16
    fp8 = ml_dtypes.float8_e4m3

    x = np.ascontiguousarray(inputs["x"], dtype=np.float32)
    cos = np.asarray(inputs["cos"], dtype=np.float32)
    sin = np.asarray(inputs["sin"], dtype=np.float32)
    c = np.asarray(inputs["c"], dtype=np.float32)

    cos_s = cos[0, :, 0, 0, :DH // 2]      # (S, 32)
    sin_s = sin[0, :, 0, 0, :DH // 2]
    # C4[p, t] = cos_s[t, p%32]; S4 sign-folded: -sin for (p%64)<32 else +sin
    pidx = np.arange(P)
    C4 = cos_s.T[pidx % 32, :]             # (128, S)
    sgn = np.where((pidx % 64) < 32, -1.0, 1.0).astype(np.float32)
    S4 = sin_s.T[pidx % 32, :] * sgn[:, None]

    WadaT = np.ascontiguousarray(inputs["W_ada"].T.astype(bf16))        # (128, 4608)
    badaT = np.ascontiguousarray(
        np.asarray(inputs["b_ada"], np.float32).reshape(36, P).T)       # (128, 36)
    def blocks(wT, nblk):
        # wT: (K, N) -> (nblk, 128, K//128, 128): block m holds lhsT tiles
        K, N = wT.shape
        return np.ascontiguousarray(
            wT.reshape(K // P, P, nblk, P).transpose(2, 1, 0, 3)).astype(bf16)

    def blocks8(wT, nblk, s):
        K, N = wT.shape
        return np.ascontiguousarray(
            (s * wT).reshape(K // P, P, nblk, P).transpose(2, 1, 0, 3)
        ).astype(fp8)

    WqkvT = inputs["W_qkv"].T.astype(np.float32)                        # (768, 2304)
    WqkB = blocks8(WqkvT[:, :2 * D], 2 * DK, 64.0)                      # (12,128,6,128)
    WvR = np.ascontiguousarray(32.0 * WqkvT[:, 2 * D:]).astype(fp8)     # (768, 768)
    WoB = blocks8(inputs["W_out"].T.astype(np.float32), DK, 64.0)
    W1B = blocks(inputs["W_mlp1"].T.astype(np.float32), MK)
    W2B = blocks(inputs["W_mlp2"].T.astype(np.float32), DK)
    b1T = np.ascontiguousarray(
        np.asarray(inputs["b_mlp1"], np.float32).reshape(MK, P).T)      # (128, 24)
    b2T = np.ascontiguousarray(
        np.asarray(inputs["b_mlp2"], np.float32).reshape(DK, P).T)      # (128, 6)
    ln1wT = np.ascontiguousarray(
        np.asarray(inputs["ln1_w"], np.float32).reshape(DK, P).T)       # (128, 6)
    ln2wT = np.ascontiguousarray(
        np.asarray(inputs["ln2_w"], np.float32).reshape(DK, P).T)

    in_maps = []
    for core in range(N_CORES):
        b, half = core // 2, core % 2
        own = slice(half * SH, half * SH + SH)
        oth = slice((1 - half) * SH, (1 - half) * SH + SH)
        xb = x[b]                                            # (S, D)
        xT = np.concatenate([xb[own].T, xb[oth].T], axis=1)  # (768, 2048) own first
        cos4 = np.concatenate([C4[:, own], C4[:, oth]], axis=1).astype(bf16)
        sin4 = np.concatenate([S4[:, own], S4[:, oth]], axis=1).astype(bf16)
        xT16 = np.ascontiguousarray(
            xT.reshape(DK, P, S).transpose(1, 0, 2)).astype(bf16)
        in_maps.append({
            "xT": np.ascontiguousarray(xT),
            "xT16": xT16,
            "cT": np.ascontiguousarray(c[b].reshape(COND, 1)),
            "cos4": np.ascontiguousarray(cos4),
            "sin4": np.ascontiguousarray(sin4),
            "WadaT": WadaT, "badaT": badaT,
            "ln1wT": ln1wT, "ln2wT": ln2wT,
            "WqkB": WqkB, "WvR": WvR, "WoB": WoB,
            "W1B": W1B, "b1T": b1T, "W2B": W2B, "b2T": b2T,
        })
    return in_maps


def _get_program():
    if "nc" not in _prog_cache:
        _prog_cache["nc"] = _build_program()
    return _prog_cache["nc"]


def kernel(**inputs):
    from concourse.bass_utils import run_bass_kernel_spmd
    nc = _get_program()
    in_maps = _host_prep(inputs)
    res = run_bass_kernel_spmd(nc, in_maps, core_ids=list(range(N_CORES)))
    out = np.empty((B, S, D), dtype=np.float32)
    for core in range(N_CORES):
        b, half = core // 2, core % 2
        out[b, half * SH:(half + 1) * SH, :] = res.results[core]["outT"].T
    return out


# revision 40
# speedup vs baseline: 1.1787x; 1.1787x over previous
"""DDiT block kernel for 8 Trainium2 NeuronCores.

Sharding: core = (batch b = core//2, seq half = core%2). Each core computes
adaLN, LN1 for all 2048 tokens of its batch, q for its own 1024 tokens,
k/v for all 2048 (redundant compute instead of a collective), rotary,
non-causal attention for its 1024 queries, out-proj, LN2, MLP.
All activations live in feature-on-partition layout; the host pre-transposes
x / weights and re-assembles the output.

v12 design (~484us; v7 baseline was ~511us):
  - fp8(e4m3) attention path: W_qkv(x64)/W_v(x32)/W_out(x64) quantized on
    the host, LN1 output h in fp8, DoubleRow matmuls pair two 128-row
    contraction tiles per instruction (2x contraction throughput) for
    q/k/v projections, AV, and out-proj. Scores stay bf16 (64-contraction,
    row-pair tile_position). Scale bookkeeping: scores come out x4096
    (folded into the exp constants), AV output is 32*o (the fp8 sweet
    spot for out-proj), out-proj result is x2048 (1/2048 folded into the
    adaLN msa gate). MLP stays bf16: fp8 there costs ~1.7e-2 rel err.
  - B/C: all four LN1-stat chains first (their ACT/DVE tails overlap the
    adaLN phase nested underneath and the projection matmuls), then
    applies, q/k/v + rope for BOTH halves. Rope-swap DMAs ride the idle
    gpsimd queue.
  - window: pure scores->exp->AV pipeline. psS bufs=3 (12KB) + psO (4KB)
    fill PSUM exactly; no passes/DRAM spills. exp alternates per kc
    between ACT (true exp -> fp8) and DVE (Schraudolph: uint8(A*x+B)
    bitcast to fp8e4; negative scores saturate to 0 = exp underflow).
    The pipeline pace is the cycle exp -> scores(+3) -> exp (~1.6us per
    kc-pair, semaphore hops included); E pairs [P,2(hh),2(kc),512] u8
    feed DoubleRow AV; vA pairs are [P,2,H,80] fp8 (pair stride and head
    offsets 16B-aligned for dual-fp8 LDWEIGHTS, col 64 = ones row giving
    the softmax denominator, 65..79 zero pad).
  - finalize(p) is emitted before p+1's first scores so its PSUM-releasing
    copies (ov: ACT hh0 / DVE hh1) land at the front of both engine
    queues; denominator row must be DMA'd to a partition-0 tile before
    reciprocal_approx_fast (ucode reads garbage at partition offset 64).
  - oTs is split per query half so the tail's out-proj(0) does not wait
    on the last finalize; the residual x loads as one [P,DK,2,512] tile
    dispatched at window start (scalar queue).
  - tail: op0, ln2(0)-stats, op1 (hides the ln2(0) ACT-sqrt chain),
    ln2(0)-apply, mlp1(0), ln2(1) (hidden under mlp1(0)), mlp1(1),
    mlp2(0), mlp2(1). Two ACT table switches total (exp->sqrt->gelu).
"""

import numpy as np
import sys

sys.path.insert(0, "/opt/trn_rl_repo")

B, S, D, H, DH = 4, 2048, 768, 12, 64
COND, MLP = 128, 3072
EPS = 1e-5
P = 128
SH = S // 2          # tokens per core (1024)
DK = D // P          # 6 feature chunks
MK = MLP // P        # 24 mlp chunks
KC = S // P          # 16 key blocks
N_CORES = 8

_prog_cache = {}


def _build_program():
    import concourse.tile as tile
    from concourse import bacc
    import concourse.mybir as mybir
    from contextlib import ExitStack

    f32 = mybir.dt.float32
    bf16 = mybir.dt.bfloat16
    fp8 = mybir.dt.float8e4
    u8 = mybir.dt.uint8
    AF = mybir.ActivationFunctionType
    OP = mybir.AluOpType
    MM8 = mybir.MatmulPerfMode.DoubleRow
    # q/k/v weights are scaled x64/x32 on the host for fp8; scores come out
    # x4096. Schraudolph exp2-bit-trick: uint8(A*x + B) bitcast fp8e4
    # approximates exp(x*0.125/4096) (head-scale and fp8 descale folded in).
    SCH_A = 8.0 * 1.4426950408889634 * 0.125 / 4096.0
    SCH_B = 7.0 * 8.0 - 0.344
    EXP_SC = 0.125 / 4096.0

    nc = bacc.Bacc("TRN2", target_bir_lowering=False, debug=False,
                   enable_asserts=False, num_devices=N_CORES)

    # ---- DRAM I/O (per-core shapes) ----
    xT_d = nc.dram_tensor("xT", [D, S], f32, kind="ExternalInput").ap()
    xT16_d = nc.dram_tensor("xT16", [P, DK, S], bf16, kind="ExternalInput").ap()
    c_d = nc.dram_tensor("cT", [COND, 1], f32, kind="ExternalInput").ap()
    cos_d = nc.dram_tensor("cos4", [P, S], bf16, kind="ExternalInput").ap()
    sin_d = nc.dram_tensor("sin4", [P, S], bf16, kind="ExternalInput").ap()
    wada_d = nc.dram_tensor("WadaT", [COND, 6 * D], bf16, kind="ExternalInput").ap()
    bada_d = nc.dram_tensor("badaT", [P, 36], f32, kind="ExternalInput").ap()
    ln1w_d = nc.dram_tensor("ln1wT", [P, DK], f32, kind="ExternalInput").ap()
    ln2w_d = nc.dram_tensor("ln2wT", [P, DK], f32, kind="ExternalInput").ap()
    wqk_d = nc.dram_tensor("WqkB", [2 * DK, P, DK, P], fp8, kind="ExternalInput").ap()
    wv_d = nc.dram_tensor("WvR", [D, D], fp8, kind="ExternalInput").ap()
    wout_d = nc.dram_tensor("WoB", [DK, P, DK, P], fp8, kind="ExternalInput").ap()
    w1_d = nc.dram_tensor("W1B", [MK, P, DK, P], bf16, kind="ExternalInput").ap()
    b1_d = nc.dram_tensor("b1T", [P, MK], f32, kind="ExternalInput").ap()
    w2_d = nc.dram_tensor("W2B", [DK, P, MK, P], bf16, kind="ExternalInput").ap()
    b2_d = nc.dram_tensor("b2T", [P, DK], f32, kind="ExternalInput").ap()
    out_d = nc.dram_tensor("outT", [D, SH], f32, kind="ExternalOutput").ap()

    xT3 = xT_d.rearrange("(a p) n -> p a n", p=P)          # [128, 6, 2048]

    with tile.TileContext(nc) as tc, ExitStack() as ctx:
        base = ctx.enter_context(tc.tile_pool(name="base", bufs=1))
        wpool = ctx.enter_context(tc.tile_pool(name="wpool", bufs=3))
        stat = ctx.enter_context(tc.tile_pool(name="stat", bufs=1))
        bcast = ctx.enter_context(tc.tile_pool(name="bcast", bufs=4))
        sqp = ctx.enter_context(tc.tile_pool(name="sqp", bufs=2))
        rp = ctx.enter_context(tc.tile_pool(name="rope", bufs=1))

        ada = base.tile([P, 36], f32, name="ada")
        ln1s = base.tile([P, DK], f32, name="ln1s")
        ln2s = base.tile([P, DK], f32, name="ln2s")
        ones = base.tile([P, 1], bf16, name="ones")
        nc.vector.memset(ones[:], 1.0)
        epsT = base.tile([1, 1], f32, name="epsT")
        nc.vector.memset(epsT[:], EPS)
        b1s = base.tile([P, MK], f32, name="b1s")
        b2s = base.tile([P, DK], f32, name="b2s")
        cosT = base.tile([P, S], bf16, name="cosT")
        sinT = base.tile([P, S], bf16, name="sinT")

        # cT + weight DMAs kick off first (scalar queue)
        cT = base.tile([COND, 1], f32, name="cT")
        nc.scalar.dma_start(cT[:], c_d[:, :])
        cT16 = base.tile([COND, 1], bf16, name="cT16")
        nc.vector.tensor_copy(cT16[:], cT[:])

        def ln_stats(psp, ps_tag, src_chunk, use_act=True, sqrt_dve=False):
            """Sums/var/rstd for 512 columns; returns (A128, B128) bf16
            broadcast tiles (rstd and mean)."""
            ps = psp.tile([P, 512], f32, tag=ps_tag, name="lnps")
            for k in range(DK):
                x16 = src_chunk(k)
                sq = sqp.tile([P, 512], bf16, tag="sq", name="sq")
                if use_act:
                    nc.scalar.activation(sq[:], x16[:], AF.Square)
                else:
                    nc.vector.tensor_mul(sq[:], x16[:], x16[:])
                nc.tensor.matmul(ps[0:1, :], ones[:], x16[:],
                                 start=(k == 0), stop=(k == DK - 1))
                nc.tensor.matmul(ps[32:33, :], ones[:], sq[:],
                                 start=(k == 0), stop=(k == DK - 1))
            mean = stat.tile([1, 512], f32, tag="mean", name="mean")
            nc.vector.tensor_scalar_mul(mean[:], ps[0:1, :], 1.0 / D)
            var = stat.tile([1, 512], f32, tag="var", name="var")
            nc.vector.tensor_scalar_mul(var[:], ps[32:33, :], 1.0 / D)
            aux = stat.tile([1, 512], f32, tag="aux", name="aux")
            nc.vector.tensor_mul(aux[:], mean[:], mean[:])
            nc.vector.tensor_sub(var[:], var[:], aux[:])
            r0 = stat.tile([1, 512], f32, tag="r0", name="r0")
            if not sqrt_dve:
                sd = stat.tile([1, 512], f32, tag="aux", name="sd")
                nc.scalar.activation(sd[:], var[:], AF.Sqrt, bias=epsT[:])
                nc.vector.reciprocal_approx_fast(out=r0[:], in_=sd[:])
            else:
                # rsqrt via clamped Newton from a constant seed (no ACT
                # sqrt -> no activation-table switches near gelu phases)
                vc = stat.tile([1, 512], f32, tag="vc", name="vc")
                nc.vector.tensor_scalar(vc[:], var[:], 10.0, EPS,
                                        OP.min, OP.add)
                nc.vector.memset(r0[:], 0.4)
                nt = stat.tile([1, 512], f32, tag="nt", name="nt")
                for _ in range(5):
                    nc.vector.tensor_mul(nt[:], r0[:], r0[:])
                    nc.vector.tensor_mul(nt[:], nt[:], vc[:])
                    nc.vector.tensor_scalar(nt[:], nt[:], -0.5, 1.5,
                                            OP.mult, OP.add)
                    nc.vector.tensor_mul(r0[:], r0[:], nt[:])
            rb16 = stat.tile([1, 512], bf16, tag="rb16", name="rb16")
            nc.vector.tensor_copy(rb16[:], r0[:])
            mb16 = stat.tile([1, 512], bf16, tag="mb16", name="mb16")
            nc.vector.tensor_copy(mb16[:], mean[:])
            A128 = bcast.tile([P, 512], bf16, tag="A128", name="A128")
            B128 = bcast.tile([P, 512], bf16, tag="B128", name="B128")
            nc.gpsimd.partition_broadcast(A128[:], rb16[:])
            nc.gpsimd.partition_broadcast(B128[:], mb16[:])
            return A128, B128

        def ln_apply(src_chunk, A128, B128, scale_cols, shift_col0, dst_chunk,
                     use_act=True):
            for k in range(DK):
                t2 = sqp.tile([P, 512], bf16, tag="t2", name="t2")
                nc.vector.tensor_sub(t2[:], src_chunk(k), B128[:])
                nc.vector.tensor_mul(t2[:], t2[:], A128[:])
                if use_act:
                    nc.scalar.activation(
                        dst_chunk(k), t2[:], AF.Identity,
                        bias=ada[:, shift_col0 + k:shift_col0 + k + 1],
                        scale=scale_cols[:, k:k + 1])
                else:
                    nc.vector.tensor_scalar(
                        dst_chunk(k), t2[:], scale_cols[:, k:k + 1],
                        ada[:, shift_col0 + k:shift_col0 + k + 1],
                        OP.mult, OP.add)

        def ln_block(psp, ps_tag, src_chunk, scale_cols, shift_col0, dst_chunk,
                     use_act=True, sqrt_dve=False):
            A128, B128 = ln_stats(psp, ps_tag, src_chunk, use_act=use_act,
                                  sqrt_dve=sqrt_dve)
            ln_apply(src_chunk, A128, B128, scale_cols, shift_col0, dst_chunk,
                     use_act=use_act)

        with tc.tile_pool(name="efgA", bufs=1) as efgA:
            # ======== q/k/v outputs (live through attention) ========
            with tc.tile_pool(name="qkv_out", bufs=1) as qko:
                qT = [qko.tile([P, SH], bf16, name=f"qT{m}") for m in range(DK)]
                kpair = [qko.tile([P, S], bf16, name=f"kp{m}")
                         for m in range(DK)]
                # vA pairs for DoubleRow AV: [keys, kc-pair, head, DH+pad16]
                vA8 = [qko.tile([P, 2, H, 80], fp8, name=f"vA{tp}")
                       for tp in range(KC // 2)]
                wvT = qko.tile([P, DK, D], fp8, name="wvT")

                def rope_swap(sw, src, n):
                    # gpsimd queue: keeps the 48 dispatches off the busy
                    # sync queue in B/C
                    nc.gpsimd.dma_start(sw[0:32, 0:n], src[32:64, 0:n])
                    nc.gpsimd.dma_start(sw[32:64, 0:n], src[0:32, 0:n])
                    nc.gpsimd.dma_start(sw[64:96, 0:n], src[96:128, 0:n])
                    nc.gpsimd.dma_start(sw[96:128, 0:n], src[64:96, 0:n])

                def rope_q(m):
                    sw = rp.tile([P, SH], bf16, tag="qsw", name="qsw")
                    t = qT[m]
                    rope_swap(sw, t[:, 0:SH], SH)
                    nc.vector.tensor_mul(t[:], t[:], cosT[:, 0:SH])
                    nc.vector.tensor_mul(sw[:], sw[:], sinT[:, 0:SH])
                    nc.vector.tensor_add(t[:], t[:], sw[:])

                def rope_k(m, b2):
                    sl = slice(b2 * SH, b2 * SH + SH)
                    sw = rp.tile([P, SH], bf16, tag="ksw", name="ksw")
                    t = kpair[m]
                    rope_swap(sw, t[:, sl], SH)
                    nc.vector.tensor_mul(t[:, sl], t[:, sl], cosT[:, sl])
                    nc.vector.tensor_mul(sw[:], sw[:], sinT[:, sl])
                    nc.vector.tensor_add(t[:, sl], t[:, sl], sw[:])

                def v_proj(t, hbsrc):
                    """v for token block t (128 tokens) from hb half tiles."""
                    ps = psQ.tile([P, 2, 512], f32, tag="mm", name="ps_v")
                    tl = t % 8
                    tp, sub = t // 2, t % 2
                    for j in range(3):
                        lhs = hbsrc[tl // 4][:, 2 * j:2 * j + 2,
                                             (tl % 4) * P:(tl % 4 + 1) * P]
                        nc.tensor.matmul(ps[:, 0, :], lhs,
                                         wvT[:, 2 * j:2 * j + 2, 0:512],
                                         start=(j == 0), stop=(j == 2),
                                         perf_mode=MM8)
                        nc.tensor.matmul(ps[:, 1, 0:256], lhs,
                                         wvT[:, 2 * j:2 * j + 2, 512:768],
                                         start=(j == 0), stop=(j == 2),
                                         perf_mode=MM8)
                    nc.scalar.copy(
                        vA8[tp][:, sub, 0:8, 0:DH],
                        ps[:, 0, :].rearrange("p (h d) -> p h d", d=DH))
                    nc.vector.tensor_copy(
                        vA8[tp][:, sub, 8:H, 0:DH],
                        ps[:, 1, 0:256].rearrange("p (h d) -> p h d", d=DH))
                    nc.vector.memset(vA8[tp][:, sub, :, DH:DH + 1], 1.0)
                    nc.vector.memset(vA8[tp][:, sub, :, DH + 1:80], 0.0)

                # ==== Phase B/C: LN1 + q/k/v + rope for both halves ====
                with tc.tile_pool(name="phbc", bufs=4) as phbc, \
                     tc.tile_pool(name="hbp", bufs=4) as hbp, \
                     tc.tile_pool(name="psLN", bufs=2, space="PSUM") as psLN, \
                     tc.tile_pool(name="psQ", bufs=2, space="PSUM") as psQ:
                    adaw_ctx = tc.tile_pool(name="adaw", bufs=1)
                    adaw = adaw_ctx.__enter__()
                    wt = adaw.tile([COND, 6 * D], bf16, name="wadaT")
                    nc.scalar.dma_start(wt[:], wada_d[:, :])
                    # all 4 LN1 stats first; their ACT/DVE chains overlap
                    # phase A and the projection matmuls below
                    xbs, ABs = [], []
                    for i in range(4):
                        xb = phbc.tile([P, DK, 512], bf16, tag="xb",
                                       name=f"xb{i}")
                        nc.sync.dma_start(xb[:], xT16_d[:, :, i * 512:
                                                        i * 512 + 512])
                        xbs.append(xb)
                        ABs.append(ln_stats(psLN, "lnps",
                                            lambda k, xb=xb: xb[:, k, :]))
                        if i == 0:
                            nc.scalar.dma_start(cosT[:], cos_d[:, :])
                            nc.scalar.dma_start(sinT[:], sin_d[:, :])
                            nc.scalar.dma_start(b1s[:], b1_d[:, :])
                            nc.scalar.dma_start(b2s[:], b2_d[:, :])

                    # ---- Phase A: adaLN modulation (hidden under LN chains)
                    with tc.tile_pool(name="psE", bufs=2, space="PSUM") as psE:
                        for j4 in range(9):
                            ps = psE.tile([P, 4], f32, tag="mm", name="ps_ada")
                            for j in range(4):
                                nc.tensor.matmul(ps[:, j:j + 1],
                                                 wt[:, (4 * j4 + j) * P:
                                                     (4 * j4 + j + 1) * P],
                                                 cT16[:],
                                                 start=True, stop=True)
                            nc.vector.tensor_copy(ada[:, 4 * j4:4 * j4 + 4],
                                                  ps[:])
                        badaT = base.tile([P, 36], f32, name="badaT")
                        nc.scalar.dma_start(badaT[:], bada_d[:, :])
                        nc.vector.tensor_add(ada[:], ada[:], badaT[:])
                        nc.vector.tensor_scalar_add(ada[:, 6:12],
                                                    ada[:, 6:12], 1.0)
                        nc.vector.tensor_scalar_add(ada[:, 24:30],
                                                    ada[:, 24:30], 1.0)
                        lw = base.tile([P, DK], f32, name="lnw1")
                        nc.scalar.dma_start(lw[:], ln1w_d[:, :])
                        nc.vector.tensor_mul(ln1s[:], lw[:], ada[:, 6:12])
                        lw2 = base.tile([P, DK], f32, name="lnw2")
                        nc.scalar.dma_start(lw2[:], ln2w_d[:, :])
                        nc.vector.tensor_mul(ln2s[:], lw2[:], ada[:, 24:30])
                        nc.vector.tensor_scalar_mul(ada[:, 12:18],
                                                    ada[:, 12:18],
                                                    1.0 / 2048.0)
                    adaw_ctx.__exit__(None, None, None)

                    # applies for the own half, then projections (the b2=1
                    # applies run under the q/k matmuls)
                    hb, hb1 = [], []
                    for i in range(2):
                        hbt = hbp.tile([P, DK, 512], fp8, tag="hb",
                                       name=f"hb{i}")
                        hb.append(hbt)
                        ln_apply(lambda k, xb=xbs[i]: xb[:, k, :], ABs[i][0],
                                 ABs[i][1], ln1s, 0,
                                 lambda k, hbt=hbt: hbt[:, k, :])
                    for i in range(2):
                        hbt = hbp.tile([P, DK, 512], fp8, tag="hb",
                                       name=f"hb1_{i}")
                        hb1.append(hbt)
                        ln_apply(lambda k, xb=xbs[2 + i]: xb[:, k, :],
                                 ABs[2 + i][0], ABs[2 + i][1], ln1s, 0,
                                 lambda k, hbt=hbt: hbt[:, k, :])
                    for is_k, wblk0 in ((0, 0), (1, DK)):
                        for m in range(DK):
                            w6 = wpool.tile([P, DK, P], fp8, tag="w6",
                                            name="w6")
                            nc.sync.dma_start(w6[:], wqk_d[wblk0 + m])
                            ps = psQ.tile([P, 2, 512], f32, tag="mm",
                                          name="ps_qk")
                            for j in range(3):
                                for i in range(2):
                                    nc.tensor.matmul(
                                        ps[:, i, :],
                                        w6[:, 2 * j:2 * j + 2, :],
                                        hb[i][:, 2 * j:2 * j + 2, :],
                                        start=(j == 0), stop=(j == 2),
                                        perf_mode=MM8)
                            if is_k:
                                nc.scalar.copy(
                                    kpair[m][:, 0:SH],
                                    ps[:].rearrange("p a n -> p (a n)"))
                            else:
                                nc.vector.tensor_copy(
                                    qT[m][:, 0:SH],
                                    ps[:].rearrange("p a n -> p (a n)"))
                            if is_k:
                                rope_k(m, 0)
                            else:
                                rope_q(m)
                        if is_k:
                            nc.sync.dma_start(
                                wvT[:],
                                wv_d.rearrange("(a p) n -> p a n", p=P))
                    for t in range(SH // P):
                        v_proj(t, hb)

                    # ---- other half (b2=1): k/v proj + rope ----
                    for m in range(DK):
                        w6 = wpool.tile([P, DK, P], fp8, tag="w6", name="w6")
                        nc.sync.dma_start(w6[:], wqk_d[DK + m])
                        ps = psQ.tile([P, 2, 512], f32, tag="mm", name="ps_k1")
                        for j in range(3):
                            for i in range(2):
                                nc.tensor.matmul(ps[:, i, :],
                                                 w6[:, 2 * j:2 * j + 2, :],
                                                 hb1[i][:, 2 * j:2 * j + 2, :],
                                                 start=(j == 0), stop=(j == 2),
                                                 perf_mode=MM8)
                        nc.scalar.copy(
                            kpair[m][:, SH:S],
                            ps[:].rearrange("p a n -> p (a n)"))
                        rope_k(m, 1)
                    for t in range(8, KC):
                        v_proj(t, hb1)

                # ==== Phase D: attention (pure pipeline) ====
                xoall = efgA.tile([P, DK, 2, 512], f32, name="xoall")
                nc.scalar.dma_start(
                    xoall[:],
                    xT3[:, :, 0:SH].rearrange("p a (b n) -> p a b n", n=512))
                with tc.tile_pool(name="attn_sb", bufs=4) as asb, \
                     tc.tile_pool(name="fin", bufs=2) as fin, \
                     tc.tile_pool(name="fin1", bufs=1) as fin1, \
                     tc.tile_pool(name="psS", bufs=3, space="PSUM") as psS, \
                     tc.tile_pool(name="psO", bufs=1, space="PSUM") as psO:
                    oTs0 = efgA.tile([P, DK, 512], fp8, name="oTs0")
                    oTs1 = efgA.tile([P, DK, 512], fp8, name="oTs1")
                    oTsq = [oTs0, oTs1]

                    def emit_scores(p, qb, kc, Ep):
                        sg = psS.tile([P, 2, 512], f32, tag="sg", name="sg")
                        qsl = slice(qb * 512, qb * 512 + 512)
                        for hh in range(2):
                            r0_, r1_ = 64 * hh, 64 * hh + 64
                            nc.tensor.matmul(
                                sg[:, hh, :],
                                kpair[p][r0_:r1_, kc * P:(kc + 1) * P],
                                qT[p][r0_:r1_, qsl], start=True, stop=True)
                        # alternate exp engine per kc: consecutive exps run
                        # concurrently on ACT and DVE.
                        out_ap = Ep[:, :, kc % 2, :]
                        if kc % 2 == 0:
                            nc.scalar.activation(out_ap.bitcast(fp8), sg[:],
                                                 AF.Exp, scale=EXP_SC)
                        else:
                            nc.vector.tensor_scalar(out_ap, sg[:], SCH_A,
                                                    SCH_B, OP.mult, OP.add)

                    def emit_av(p, qb, kcp, Ep, oags):
                        for hh in range(2):
                            nc.tensor.matmul(oags[hh][:],
                                             vA8[kcp][:, :, 2 * p + hh, :],
                                             Ep[:, hh, :, :].bitcast(fp8),
                                             start=(kcp == 0),
                                             stop=(kcp == KC // 2 - 1),
                                             perf_mode=MM8)

                    def emit_finalize(p, qb, oags):
                        oTs = oTsq[qb]
                        ov = fin.tile([DH + 1, 2, 512], f32, tag="ov",
                                      name="ov")
                        nc.scalar.copy(ov[:, 0, :], oags[0][0:DH + 1, :])
                        nc.vector.tensor_copy(ov[:, 1, :],
                                              oags[1][0:DH + 1, :])
                        dn = fin1.tile([1, 2, 512], f32, tag="dn", name="dn")
                        nc.sync.dma_start(dn[:], ov[DH:DH + 1, :, :])
                        rc = fin1.tile([1, 2, 512], f32, tag="rc", name="rc")
                        nc.vector.reciprocal_approx_fast(out=rc[:], in_=dn[:])
                        rcb = fin1.tile([1, 2, 512], bf16, tag="rcb",
                                        name="rcb")
                        nc.vector.tensor_copy(rcb[:], rc[:])
                        rb = fin1.tile([DH, 2, 512], bf16, tag="rb", name="rb")
                        nc.gpsimd.partition_broadcast(rb[:], rcb[:])
                        nc.vector.tensor_mul(oTs[0:DH, p, :],
                                             ov[0:DH, 0, :], rb[:, 0, :])
                        ot = fin1.tile([DH, 512], fp8, tag="ot", name="ot")
                        nc.vector.tensor_mul(ot[:], ov[0:DH, 1, :],
                                             rb[:, 1, :])
                        nc.sync.dma_start(oTs[DH:P, p, :], ot[:])

                    prevfin = None
                    pend = None
                    for qb in range(2):
                        for p in range(H // 2):
                            oags = [psO.tile([80, 512], f32,
                                             tag=f"oag{hh}", name=f"oag{hh}")
                                    for hh in range(2)]
                            if prevfin is not None:
                                emit_finalize(*prevfin)
                                prevfin = None
                            for kcp in range(KC // 2):
                                Ep = asb.tile([P, 2, 2, 512], u8, tag="E",
                                              name="E")
                                emit_scores(p, qb, 2 * kcp, Ep)
                                emit_scores(p, qb, 2 * kcp + 1, Ep)
                                if pend is not None:
                                    emit_av(*pend)
                                pend = (p, qb, kcp, Ep, oags)
                            prevfin = (p, qb, oags)
                    emit_av(*pend)
                    emit_finalize(*prevfin)

            # ======== EFG tail ========
            with tc.tile_pool(name="efgB", bufs=1) as efgB, \
                 tc.tile_pool(name="mlp_tmp", bufs=2) as mt, \
                 tc.tile_pool(name="w24p", bufs=2) as w24p, \
                 tc.tile_pool(name="psM", bufs=2, space="PSUM") as psM:
                x1_0 = efgB.tile([P, DK, 512], bf16, name="x1_0")
                h2_0 = efgB.tile([P, DK, 512], bf16, name="h2_0")
                x1_1 = efgB.tile([P, DK, 512], bf16, name="x1_1")
                h2_1 = efgB.tile([P, DK, 512], bf16, name="h2_1")
                m16_0 = efgB.tile([P, MK, 512], bf16, name="m16_0")
                m16_1 = efgB.tile([P, MK, 512], bf16, name="m16_1")

                def op_unit(ihalf, m, x1t):
                    w6 = wpool.tile([P, DK, P], fp8, tag="w6o", name="w6o")
                    nc.sync.dma_start(w6[:], wout_d[m])
                    ps = psM.tile([P, 512], f32, tag="mm2", name="ps_o")
                    for j in range(3):
                        nc.tensor.matmul(ps[:],
                                         w6[:, 2 * j:2 * j + 2, :],
                                         oTsq[ihalf][:, 2 * j:2 * j + 2, :],
                                         start=(j == 0), stop=(j == 2),
                                         perf_mode=MM8)
                    nc.vector.scalar_tensor_tensor(
                        x1t[:, m, :], ps[:], ada[:, 12 + m:13 + m],
                        xoall[:, m, ihalf, :], OP.mult, OP.add)

                def mlp1_unit(m, h2t, m16t):
                    w6 = wpool.tile([P, DK, P], bf16, tag="w6m", name="w6m")
                    nc.sync.dma_start(w6[:], w1_d[m])
                    ps = psM.tile([P, 512], f32, tag="mm2", name="ps_m")
                    for k in range(DK):
                        nc.tensor.matmul(ps[:], w6[:, k, :], h2t[:, k, :],
                                         start=(k == 0), stop=(k == DK - 1))
                    nc.scalar.activation(m16t[:, m, :], ps[:],
                                         AF.Gelu_apprx_tanh,
                                         bias=b1s[:, m:m + 1])

                def mlp2_half(m, i, m16t, x1t):
                    isl = slice(i * 512, i * 512 + 512)
                    w24 = w24p.tile([P, MK, P], bf16, tag="w24", name="w24")
                    nc.sync.dma_start(w24[:], w2_d[m])
                    ps = psM.tile([P, 512], f32, tag="mm2", name="ps_y")
                    for k in range(MK):
                        nc.tensor.matmul(ps[:], w24[:, k, :], m16t[:, k, :],
                                         start=(k == 0), stop=(k == MK - 1))
                    yt = mt.tile([P, 512], f32, tag="yt", name="yt")
                    nc.vector.tensor_scalar(yt[:], ps[:], b2s[:, m:m + 1],
                                            ada[:, 30 + m:31 + m],
                                            OP.add, OP.mult)
                    nc.vector.tensor_add(yt[:], yt[:], x1t[:, m, :])
                    nc.sync.dma_start(out_d[m * P:(m + 1) * P, isl], yt[:])

                for m in range(DK):
                    op_unit(0, m, x1_0)
                AB0 = ln_stats(psM, "lnps2", lambda k: x1_0[:, k, :])
                for m in range(DK):
                    op_unit(1, m, x1_1)
                ln_apply(lambda k: x1_0[:, k, :], AB0[0], AB0[1], ln2s, 18,
                         lambda k: h2_0[:, k, :], use_act=True)
                AB1 = ln_stats(psM, "lnps2", lambda k: x1_1[:, k, :])
                for m in range(MK):
                    mlp1_unit(m, h2_0, m16_0)
                ln_apply(lambda k: x1_1[:, k, :], AB1[0], AB1[1], ln2s, 18,
                         lambda k: h2_1[:, k, :], use_act=True)
                for m in range(MK):
                    mlp1_unit(m, h2_1, m16_1)
                for m in range(DK):
                    mlp2_half(m, 0, m16_0, x1_0)
                for m in range(DK):
                    mlp2_half(m, 1, m16_1, x1_1)

    nc.compile()
    return nc


def _host_prep(inputs):
    """Build per-core in_maps (host-side sharding + layout transforms)."""
    import ml_dtypes
    bf16 = ml_dtypes.bfloat16
    fp8 = ml_dtypes.float8_e4m3

    x = np.ascontiguousarray(inputs["x"], dtype=np.float32)
    cos = np.asarray(inputs["cos"], dtype=np.float32)
    sin = np.asarray(inputs["sin"], dtype=np.float32)
    c = np.asarray(inputs["c"], dtype=np.float32)

    cos_s = cos[0, :, 0, 0, :DH // 2]      # (S, 32)
    sin_s = sin[0, :, 0, 0, :DH // 2]
    # C4[p, t] = cos_s[t, p%32]; S4 sign-folded: -sin for (p%64)<32 else +sin
    pidx = np.arange(P)
    C4 = cos_s.T[pidx % 32, :]             # (128, S)
    sgn = np.where((pidx % 64) < 32, -1.0, 1.0).astype(np.float32)
    S4 = sin_s.T[pidx % 32, :] * sgn[:, None]

    WadaT = np.ascontiguousarray(inputs["W_ada"].T.astype(bf16))        # (128, 4608)
    badaT = np.ascontiguousarray(
        np.asarray(inputs["b_ada"], np.float32).reshape(36, P).T)       # (128, 36)
    def blocks(wT, nblk):
        # wT: (K, N) -> (nblk, 128, K//128, 128): block m holds lhsT tiles
        K, N = wT.shape
        return np.ascontiguousarray(
            wT.reshape(K // P, P, nblk, P).transpose(2, 1, 0, 3)).astype(bf16)

    def blocks8(wT, nblk, s):
        K, N = wT.shape
        return np.ascontiguousarray(
            (s * wT).reshape(K // P, P, nblk, P).transpose(2, 1, 0, 3)
        ).astype(fp8)

    WqkvT = inputs["W_qkv"].T.astype(np.float32)                        # (768, 2304)
    WqkB = blocks8(WqkvT[:, :2 * D], 2 * DK, 64.0)                      # (12,128,6,128)
    WvR = np.ascontiguousarray(32.0 * WqkvT[:, 2 * D:]).astype(fp8)     # (768, 768)
    WoB = blocks8(inputs["W_out"].T.astype(np.float32), DK, 64.0)
    W1B = blocks(inputs["W_mlp1"].T.astype(np.float32), MK)
    W2B = blocks(inputs["W_mlp2"].T.astype(np.float32), DK)
    b1T = np.ascontiguousarray(
        np.asarray(inputs["b_mlp1"], np.float32).reshape(MK, P).T)      # (128, 24)
    b2T = np.ascontiguousarray(
        np.asarray(inputs["b_mlp2"], np.float32).reshape(DK, P).T)      # (128, 6)
    ln1wT = np.ascontiguousarray(
        np.asarray(inputs["ln1_w"], np.float32).reshape(DK, P).T)       # (128, 6)
    ln2wT = np.ascontiguousarray(
        np.asarray(inputs["ln2_w"], np.float32).reshape(DK, P).T)

    in_maps = []
    for core in range(N_CORES):
        b, half = core // 2, core % 2
        own = slice(half * SH, half * SH + SH)
        oth = slice((1 - half) * SH, (1 - half) * SH + SH)
        xb = x[b]                                            # (S, D)
        xT = np.concatenate([xb[own].T, xb[oth].T], axis=1)  # (768, 2048) own first
        cos4 = np.concatenate([C4[:, own], C4[:, oth]], axis=1).astype(bf16)
        sin4 = np.concatenate([S4[:, own], S4[:, oth]], axis=1).astype(bf16)
        xT16 = np.ascontiguousarray(
            xT.reshape(DK, P, S).transpose(1, 0, 2)).astype(bf16)
        in_maps.append({
            "xT": np.ascontiguousarray(xT),
            "xT16": xT16,
            "cT": np.ascontiguousarray(c[b].reshape(COND, 1)),
            "cos4": np.ascontiguousarray(cos4),
            "sin4": np.ascontiguousarray(sin4),
            "WadaT": WadaT, "badaT": badaT,
            "ln1wT": ln1wT, "ln2wT": ln2wT,
            "WqkB": WqkB, "WvR": WvR, "WoB": WoB,
            "W1B": W1B, "b1T": b1T, "W2B": W2B, "b2T": b2T,
        })
    return in_maps


def _get_program():
    if "nc" not in _prog_cache:
        _prog_cache["nc"] = _build_program()
    return _prog_cache["nc"]


def kernel(**inputs):
    from concourse.bass_utils import run_bass_kernel_spmd
    nc = _get_program()
    in_maps = _host_prep(inputs)
    res = run_bass_kernel_spmd(nc, in_maps, core_ids=list(range(N_CORES)))
    out = np.empty((B, S, D), dtype=np.float32)
    for core in range(N_CORES):
        b, half = core // 2, core % 2
        out[b, half * SH:(half + 1) * SH, :] = res.results[core]["outT"].T
    return out
